# revision 74
# baseline (speedup 1.0000x reference)
"""Trainium2 Bass kernel for batched single-head attention with seq-sum pooling.

Reference computation (B=16, S=2048, D=512, fp32):
    q = x @ W_q ; k = x @ W_k ; v = x @ W_v          per batch  [S, D]
    scores = q @ k.T / sqrt(D)                        [S, S]
    attn = softmax(scores, axis=-1)
    out_b = sum_s (attn @ v)[s, :]                    [D]

Algebraic restructures (same as the bf16 version):
1. The final sum over query positions commutes through both trailing
   matmuls: out_b = ((r^T E) @ x) @ W_v, where E = exp(scores/sqrt(D)) and
   r[q] = 1/rowsum_q(E) — removes the attention-value matmul AND the V
   projection.
2. scores = x M x^T with M = W_q W_k^T computed ONCE per core — replaces both
   per-batch Q/K projections with a single G = x M projection.

FP8 acceleration (this version): the three dominant matmul phases — the
G = X M projection, the scores matmul G X^T, and the E-colsum — run in
fp8e4 with perf_mode=DoubleRow (2 k-tiles contracted per PE pass, ~1.5-2x).
Numerics handled by three exact compensations:
  - M is pre-scaled by 8 so its entries (std ~0.044) clear the fp8e4
    denormal cutoff (2^-6); the exp scale absorbs the 1/8.
  - The exp gets bias = -2 (E' = E*e^-2) so E' stays below fp8e4's 240
    ceiling (raw E can reach e^6 ~ 400 -> Inf/NaN in fp8); rowsums are
    accumulated over E' so r' = 1/rowsum' is self-consistent.
  - r' (~2e-3) is below fp8 denormal range, so the colsum stationary holds
    128*r' and the 1/128 is divided back out in the w-vector copy.
The final matvec y = w @ X and epilogue y @ W_v stay bf16: their contractions
are zero-mean, so fp8 element noise does NOT average out there (simulated
4e-2 rel err vs 6e-3 with bf16 matvec).  Simulated end-to-end rel err of this
exact quantization layout: ~6e-3 (tolerance 2e-2).

Scalar-engine layout: exp runs as [128, 2, 512] two-PSUM-bank activations
(amortizes ACT's ~245ns fixed cost + halves the 187ns accumulator reads).
The scores phases are ACT-bound (~2.45us/q-tile vs ~2.16us of PE work), so
each pair's colsum matmuls are deferred into the next pair's exp-latency
windows and batch 1's first transpose units fill the final exp->r chain.
PSUM: scores 2 bufs x 2 banks + colsum accumulator 4 banks; proj phases
rotate single-bank VIEWS of long-lived pair allocs (PSUM deps resolve per
bank, so view rings give real slack where alloc rotation does not); the
w-phase (y matvec / epilogue / row->col pads) reuses the colsum banks after
their readout, and the last batch's pads pipeline across all 4 scores banks.

Measured (8 cores, full clock): ~175 us, rel err 7.8e-3 (tolerance 2e-2);
bf16 baseline was ~240 us.  Note the part P0-downclocks to ~5/6 frequency
under sustained load, which shows up as ~1.2x on every engine's slice times.

Sharding: pure data parallelism over batch — 2 batch elements per core on 8
NeuronCores, weights replicated, no collectives.  Host concatenates per-core
[2, D] outputs.
"""

import sys

sys.path.insert(0, "/opt/trn_rl_repo")

import numpy as np

import concourse.bass as bass
import concourse.mybir as mybir
import concourse.tile as tile
from concourse import bacc
from concourse.bass_utils import run_bass_kernel_spmd

B, S, D = 16, 2048, 512
P = 128
N_CORES = 8
B_PER_CORE = B // N_CORES  # 2
SCALE = 1.0 / float(np.sqrt(D))

F32 = mybir.dt.float32
BF16 = mybir.dt.bfloat16
F8 = mybir.dt.float8e4
DR = mybir.MatmulPerfMode.DoubleRow

N_ST = S // P  # 16 s-tiles (partition tiles of the sequence dim)
N_DT = D // P  # 4 d-tiles (partition tiles of the feature dim)
NCH = 512  # moving free dim per matmul (one fp32 PSUM bank)
N_SC = S // NCH  # 4 s-chunks of the sequence dim
N_KC = S // NCH  # 4 k-chunks of the key dim

MSCALE = 8.0  # M pre-scale (keeps M entries out of fp8 denormals)
EXP_SCALE = SCALE / MSCALE
EXP_BIAS = -2.0  # keeps E' = E*e^-2 below fp8e4's 240 ceiling
CR = 128.0  # r' = CR/rowsum' fits fp8 normal range
INV_CR = 1.0 / CR


def build_nc():
    nc = bacc.Bacc("TRN2", target_bir_lowering=False, debug=False, num_devices=N_CORES)
    x_ext = nc.dram_tensor(
        "inputs", [B_PER_CORE, S, D], F32, kind="ExternalInput"
    ).ap()
    wq_ext = nc.dram_tensor("W_q", [D, D], F32, kind="ExternalInput").ap()
    wk_ext = nc.dram_tensor("W_k", [D, D], F32, kind="ExternalInput").ap()
    wv_ext = nc.dram_tensor("W_v", [D, D], F32, kind="ExternalInput").ap()
    out_ext = nc.dram_tensor("out", [B_PER_CORE, D], F32, kind="ExternalOutput").ap()

    with tile.TileContext(nc) as tc:
        with (
            tc.tile_pool(name="const", bufs=1) as const_pool,
            tc.tile_pool(name="w", bufs=1) as w_pool,
            tc.tile_pool(name="xnat", bufs=2) as xnat_pool,
            tc.tile_pool(name="xt", bufs=2) as xt_pool,
            tc.tile_pool(name="qkv", bufs=2) as qkv_pool,
            tc.tile_pool(name="e", bufs=4) as e_pool,
            tc.tile_pool(name="soft", bufs=4) as soft_pool,
            tc.tile_pool(name="wvec", bufs=2) as wvec_pool,
            tc.tile_pool(name="scps", bufs=2, space="PSUM") as sc_psum,
            tc.tile_pool(name="wps", bufs=1, space="PSUM") as w_psum,
        ):
            # Identity via one gpsimd iota (value = p - f) + DVE compare:
            # ~1us vs ~6us for the memset+affine_select path, and it clears
            # the gpsimd queue quickly so SWDGE descriptor generation for the
            # x/w loads starts immediately.
            iota_pf = const_pool.tile([P, P], mybir.dt.int32)
            nc.gpsimd.iota(
                iota_pf[:], pattern=[[-1, P]], base=0, channel_multiplier=1
            )
            ident_i = const_pool.tile([P, P], mybir.dt.int32)
            nc.vector.tensor_scalar(
                ident_i[:], iota_pf[:], 0, None, op0=mybir.AluOpType.is_equal
            )
            ident = const_pool.tile([P, P], BF16)
            nc.vector.tensor_copy(ident[:], ident_i[:])
            one_t = const_pool.tile([1, 1], BF16)
            nc.vector.memset(one_t[:], 1.0)
            expbias_t = const_pool.tile([P, 1], F32)
            nc.vector.memset(expbias_t[:], EXP_BIAS)

            # x arrives via SWDGE cast-DMA (f32 -> bf16) into natural-layout
            # staging tiles; the transpose to xT happens on the PE as a
            # REGULAR identity matmul (pipelines weight loads, counts as PE
            # activity for the HAM clock gate, doesn't serialize the DMA
            # subsystem the way crossbar transposes do).
            def dma_x_chunk(b, sc, xnat_s):
                nc.gpsimd.dma_start(
                    out=xnat_s[:, sc * 4 : (sc + 1) * 4, :],
                    in_=x_ext[b, sc * NCH : (sc + 1) * NCH, :].rearrange(
                        "(t p) d -> p t d", p=P
                    ),
                )

            w_tiles = {}

            def dma_w(name, ext):
                w_s = w_pool.tile([P, N_DT, D], BF16, tag=name)
                nc.gpsimd.dma_start(
                    out=w_s[:], in_=ext.rearrange("(t p) e -> p t e", p=P)
                )
                w_tiles[name] = w_s

            # Batch 0's x chunks and the weight loads share the SWDGE queue;
            # order so each lands just before the PE needs it.  s-tile 0 gets
            # its own small cast-DMA so the first transposes start ~2us
            # earlier than waiting for the full first chunk.
            xnat0_s = xnat_pool.tile([P, N_ST, D], BF16, tag="xnat")
            x0_loaded = [False] * N_SC
            nc.gpsimd.dma_start(
                out=xnat0_s[:, 0:1, :],
                in_=x_ext[0, 0:P, :].rearrange("(t p) d -> p t d", p=P),
            )
            nc.gpsimd.dma_start(
                out=xnat0_s[:, 1:4, :],
                in_=x_ext[0, P:NCH, :].rearrange("(t p) d -> p t d", p=P),
            )
            x0_loaded[0] = True
            dma_w("wk", wk_ext)
            dma_w("wq", wq_ext)
            dma_x_chunk(0, 1, xnat0_s)
            x0_loaded[1] = True
            dma_x_chunk(0, 2, xnat0_s)
            x0_loaded[2] = True
            dma_x_chunk(0, 3, xnat0_s)
            x0_loaded[3] = True
            dma_w("wv", wv_ext)
            wk_s, wq_s, wv_s = w_tiles["wk"], w_tiles["wq"], w_tiles["wv"]

            def sc_pair():
                """One 2-bank PSUM pair from the scores pool."""
                pair_t = sc_psum.tile([P, 2, NCH], F32, tag="sc", name="sc_pair")
                return pair_t

            def bank_ring(with_w=False):
                """A rotation of single PSUM banks, carved as VIEWS of
                long-lived allocations.  PSUM dependencies resolve per bank
                (region), so slot k's writer only waits on slot k's previous
                reader, whereas rotating whole pair ALLOCS stalls on every
                reader of the buffer (too little slack to hide the copy
                latency).  `with_w=True` adds the 4 colsum banks — legal only
                while no scores phase is live — for an 8-deep ring.  Lazy
                alloc so the pool rotation engages at first use, not at
                thunk-build time."""
                state = {"slots": None, "i": 0}

                def nxt():
                    if state["slots"] is None:
                        ring_a = sc_psum.tile(
                            [P, 2, NCH], F32, tag="sc", name="ring_a"
                        )
                        ring_b = sc_psum.tile(
                            [P, 2, NCH], F32, tag="sc", name="ring_b"
                        )
                        state["slots"] = [
                            ring_a[:, 0, :],
                            ring_a[:, 1, :],
                            ring_b[:, 0, :],
                            ring_b[:, 1, :],
                        ]
                        if with_w:
                            ring_w = w_psum.tile(
                                [P, N_KC, NCH], F32, tag="w", name="ring_w"
                            )
                            state["slots"] = [
                                state["slots"][0],
                                state["slots"][1],
                                ring_w[:, 0, :],
                                ring_w[:, 1, :],
                                state["slots"][2],
                                state["slots"][3],
                                ring_w[:, 2, :],
                                ring_w[:, 3, :],
                            ]
                    s = state["slots"][state["i"] % len(state["slots"])]
                    state["i"] += 1
                    return s

                return nxt

            # One-time prework: scores = X M X^T with M = W_q W_k^T.  M is
            # computed in bf16 (fp32 accum) and stored as fp8 * MSCALE.
            wqT_s = w_pool.tile([P, N_DT, D], BF16, tag="wqT")
            wkT_s = w_pool.tile([P, N_DT, D], BF16, tag="wkT")
            m_s = w_pool.tile([P, N_DT, D], F8, tag="m")

            def m_prework_thunks(ring):
                thunks = []

                def make_wtrans_unit(src_w, dst, t_e):
                    def th():
                        tp = ring()
                        for t_a in range(N_DT):
                            nc.tensor.matmul(
                                tp[:, t_a * P : (t_a + 1) * P],
                                src_w[:, t_a, t_e * P : (t_e + 1) * P],
                                ident[:],
                                start=True,
                                stop=True,
                                skip_group_check=True,
                            )
                        nc.vector.tensor_copy(
                            dst[:, t_e, :],
                            tp[:],
                        )

                    return th

                def make_m_group(t_a):
                    def th():
                        mp = ring()
                        for t_e in range(N_DT):
                            nc.tensor.matmul(
                                mp[:],
                                wqT_s[:, t_e, t_a * P : (t_a + 1) * P],
                                wkT_s[:, t_e, :],
                                start=(t_e == 0),
                                stop=(t_e == N_DT - 1),
                                skip_group_check=True,
                            )
                        nc.scalar.mul(m_s[:, t_a, :], mp[:], MSCALE)

                    return th

                for t_e in range(N_DT):
                    thunks.append(make_wtrans_unit(wk_s, wkT_s, t_e))
                for t_e in range(N_DT):
                    thunks.append(make_wtrans_unit(wq_s, wqT_s, t_e))
                for t_a in range(N_DT):
                    thunks.append(make_m_group(t_a))
                return thunks

            # ---------- thunk builders (emission deferred for interleaving) --

            def proj_thunks(b, xnat_s, loaded, ring):
                """Transpose + G = X M projection thunks for batch b, over a
                shared 4-bank PSUM ring.  xT copies ride ACT (idle during
                proj; DVE carries the gT copies).  Units woven into a scores
                phase use DVE copies instead (ACT is the scores bottleneck)."""
                xt_s = xt_pool.tile([P, N_DT, S], F8, tag="xt")
                gt_s = qkv_pool.tile([P, N_DT, S], F8, tag="gt")

                def make_dma(sc):
                    def th():
                        dma_x_chunk(b, sc, xnat_s)

                    return th

                dma_th = [
                    None if loaded[sc] else make_dma(sc) for sc in range(N_SC)
                ]

                def make_trans_unit(st, copy_eng="scalar"):
                    def th():
                        tp = ring()
                        for dt_i in range(N_DT):
                            nc.tensor.matmul(
                                tp[:, dt_i * P : (dt_i + 1) * P],
                                xnat_s[:, st, dt_i * P : (dt_i + 1) * P],
                                ident[:],
                                start=True,
                                stop=True,
                                skip_group_check=True,
                            )
                        eng = (
                            nc.scalar.copy
                            if copy_eng == "scalar"
                            else nc.vector.tensor_copy
                        )
                        eng(
                            xt_s[:, :, st * P : (st + 1) * P],
                            tp[:].rearrange("p (t c) -> p t c", t=N_DT),
                        )

                    return th

                trans_th = [
                    [
                        make_trans_unit(
                            sc * 4 + t_i,
                            "vector" if (b == 1 and sc == 0) else "scalar",
                        )
                        for t_i in range(4)
                    ]
                    for sc in range(N_SC)
                ]

                def make_g(sc, ct):
                    def th():
                        mp = ring()
                        for h in range(2):
                            nc.tensor.matmul(
                                mp[:],
                                m_s[:, 2 * h : 2 * h + 2, ct * P : (ct + 1) * P],
                                xt_s[:, 2 * h : 2 * h + 2, sc * NCH : (sc + 1) * NCH],
                                start=(h == 0),
                                stop=(h == 1),
                                perf_mode=DR,
                                skip_group_check=True,
                            )
                        # batch 1's projection phase carries batch 0's woven
                        # w-phase on top of only 12 transpose copies, so ACT
                        # has headroom there: alternate the gT copies between
                        # DVE and ACT to balance.  Batch 0's proj keeps them
                        # all on DVE (ACT carries all 16 xT copies).
                        eng = (
                            nc.scalar.copy
                            if (b == 1 and ct % 2 == 1)
                            else nc.vector.tensor_copy
                        )
                        eng(gt_s[:, ct, sc * NCH : (sc + 1) * NCH], mp[:])

                    return th

                kq_th = [
                    [make_g(sc, ct) for ct in range(N_DT)]
                    for sc in range(N_SC)
                ]
                return (gt_s, xt_s), dma_th, trans_th, kq_th

            def emit_ltp(dma_th, trans_th, kq_th, extra=None):
                """Emit the transpose/projection stream: chunk sc+1's
                transposes weave between chunk sc's projection groups so the
                PE stream stays dense.  `extra` thunks (the previous batch's
                w-phase) are spread evenly — at most 1-2 after any unit — so
                their cross-engine latency chains never clog the PE's 4-deep
                wait queue."""
                extra = list(extra) if extra else []
                n_slots = sum(len(g) for g in kq_th) + sum(
                    len(t) for t in trans_th
                )
                prog = [0, 0]  # units emitted, extras emitted

                def slot():
                    prog[0] += 1
                    target = (prog[0] * len(extra)) // max(n_slots, 1)
                    while prog[1] < min(target, len(extra)):
                        extra[prog[1]]()
                        prog[1] += 1

                for j in (0, 1, 2):
                    if dma_th[j] is not None:
                        dma_th[j]()
                        dma_th[j] = None
                # chunks 0 AND 1's transposes go up front, then chunk sc's G
                # groups weave chunk sc+2's transposes: a G group reads ALL
                # of its chunk's xT columns, so the producing copies need a
                # full chunk of slack to land before the reader — weaving
                # only one chunk ahead left them just-in-time and stalled
                # every group.
                for th in trans_th[0] + trans_th[1]:
                    th()
                    slot()
                for sc in range(N_SC):
                    if sc + 3 < N_SC and dma_th[sc + 3] is not None:
                        dma_th[sc + 3]()
                        dma_th[sc + 3] = None
                    nxt = trans_th[sc + 2] if sc + 2 < N_SC else []
                    groups = list(kq_th[sc])
                    ti = 0
                    for g_i, g in enumerate(groups):
                        g()
                        slot()
                        while ti < len(nxt) and ti * len(groups) < (g_i + 1) * len(nxt):
                            nxt[ti]()
                            ti += 1
                            slot()
                    while ti < len(nxt):
                        nxt[ti]()
                        ti += 1
                        slot()
                while prog[1] < len(extra):
                    extra[prog[1]]()
                    prog[1] += 1

            def emit_scores_qt(gt_s, xt_s, e_pair, slot, qt, fill):
                """scores (fp8 DoubleRow) + exp for one q-tile; the exp runs
                as two [P, 2, NCH] two-bank activations with rowsum accum.
                One `fill` thunk (deferred colsum MM / woven work) is emitted
                after each half's matmuls to cover the exp-latency window of
                the two-buffer PSUM rotation."""
                rsum = soft_pool.tile([P, 2], F32, tag="rsum")
                for half in range(2):
                    sp = sc_psum.tile([P, 2, NCH], F32, tag="sc")
                    for kk in range(2):
                        kc = half * 2 + kk
                        for h in range(2):
                            nc.tensor.matmul(
                                sp[:, kk, :],
                                gt_s[:, 2 * h : 2 * h + 2, qt * P : (qt + 1) * P],
                                xt_s[:, 2 * h : 2 * h + 2, kc * NCH : (kc + 1) * NCH],
                                start=(h == 0),
                                stop=(h == 1),
                                perf_mode=DR,
                            )
                    nc.scalar.activation(
                        e_pair[:, slot, half * 2 * NCH : (half + 1) * 2 * NCH]
                        .rearrange("p (a b) -> p a b", a=2),
                        sp[:],
                        mybir.ActivationFunctionType.Exp,
                        scale=EXP_SCALE,
                        bias=expbias_t[:],
                        accum_out=rsum[:, half : half + 1],
                    )
                    if fill:
                        fill.pop(0)()
                rtot = soft_pool.tile([P, 1], F32, tag="rtot")
                nc.vector.reduce_sum(rtot[:], rsum[:], axis=mybir.AxisListType.X)
                rrec = soft_pool.tile([P, 1], F32, tag="rrec")
                nc.vector.reciprocal(rrec[:], rtot[:])
                return rrec

            def colsum_thunks(w_ps, e_pair, r_pair, pair):
                """w_ps[:, kc, :] += sum_i bcast(r_i)^T @ E_i — one fp8
                DoubleRow matmul per kc covers both q-tiles of the pair."""

                def make(kc):
                    def th():
                        nc.tensor.matmul(
                            w_ps[:, kc, :],
                            r_pair[:],
                            e_pair[:, :, kc * NCH : (kc + 1) * NCH],
                            start=(pair == 0),
                            stop=(pair == N_ST // 2 - 1),
                            perf_mode=DR,
                            skip_group_check=True,
                        )

                    return th

                return [make(kc) for kc in range(N_KC)]

            def phase_scores(b, gt_s, xt_s, per_qt_extra=None, tail_filler=None):
                w_ps = w_psum.tile([P, N_KC, NCH], F32, tag="w")
                fill = []
                for pair in range(N_ST // 2):
                    e_pair = e_pool.tile([P, 2, S], F8, tag="e")
                    r_pair = soft_pool.tile([P, 2, P], F8, tag="rp")
                    for slot in range(2):
                        qt = 2 * pair + slot
                        rrec = emit_scores_qt(gt_s, xt_s, e_pair, slot, qt, fill)
                        # r' = CR/rowsum' broadcast across a 128-wide
                        # stationary slot (fp8), on DVE
                        nc.vector.tensor_scalar_mul(
                            r_pair[:, slot, :],
                            rrec[:, 0:1].broadcast_to([P, P]),
                            CR,
                        )
                        if per_qt_extra is not None and qt < len(per_qt_extra):
                            per_qt_extra[qt]()
                    # defer each pair's colsum MMs into the next pair's
                    # exp-latency windows
                    fill.extend(colsum_thunks(w_ps, e_pair, r_pair, pair))
                # PE filler emitted BEFORE the last pair's colsum flush: the
                # flush waits ~2us on the final exp -> rowsum -> r chain, and
                # in-order PE would idle there (long enough for HAM to
                # re-throttle the clock)
                if tail_filler is not None:
                    for th in tail_filler:
                        th()
                for th in fill:
                    th()
                return w_ps

            def final_thunks(b, w_ps, xnat_s, pad_ps=None):
                """w-phase thunks, using out = (w @ X) @ W_v so no V
                projection is ever materialized.  All bf16 (fp8 noise does
                not average out in these zero-mean contractions).  PSUM for
                y / epilogue reuses the w_ps banks after their readout;
                row->col pads pipeline across `pad_ps` bank slots (PSUM
                dependencies resolve per bank, so only distinct banks give
                slack)."""
                w_sb = wvec_pool.tile([1, S], BF16, tag="wsb")
                y_ps = w_ps[:, 0, :]
                o_ps = w_ps[:, 1, :]
                if pad_ps is None:
                    pad_ps = [w_ps[:, 2, 0:1], w_ps[:, 3, 0:1]]
                wt_pads = {}
                yt_pads = {}
                thunks = []

                def make_wcopy(kc):
                    def th():
                        # divide out the CR that rode in on r'
                        if kc % 2 == 0:
                            nc.scalar.mul(
                                w_sb[:, kc * NCH : (kc + 1) * NCH],
                                w_ps[0:1, kc, :],
                                INV_CR,
                            )
                        else:
                            nc.vector.tensor_scalar_mul(
                                w_sb[:, kc * NCH : (kc + 1) * NCH],
                                w_ps[0:1, kc, :],
                                INV_CR,
                            )

                    return th

                def row_to_bcast_cols(src_row, pads, key, tag, pidx):
                    """[1,128] SBUF row chunk -> K=1 matmul -> [128,1] PSUM
                    column -> broadcast to a [128,128] stationary tile.  The
                    broadcasts alternate ACT/DVE so neither serializes the
                    w-phase (keeps the PE stream dense enough that HAM stays
                    at full clock)."""
                    tp = pad_ps[pidx]
                    nc.tensor.matmul(
                        tp,
                        src_row,
                        one_t[0:1, 0:1],
                        start=True,
                        stop=True,
                        skip_group_check=True,
                    )
                    pad = wvec_pool.tile([P, P], BF16, tag=tag)
                    eng = nc.scalar.copy if pidx % 2 == 0 else nc.vector.tensor_copy
                    eng(pad[:], tp[:, 0:1].broadcast_to([P, P]))
                    pads[key] = pad

                def make_wtrans(kt):
                    def th():
                        row_to_bcast_cols(
                            w_sb[0:1, kt * P : (kt + 1) * P],
                            wt_pads, kt, f"wtp{kt % 8}", kt % len(pad_ps),
                        )

                    return th

                def make_ymm(st):
                    def th():
                        nc.tensor.matmul(
                            y_ps,
                            wt_pads[st][:],
                            xnat_s[:, st, :],
                            start=(st == 0),
                            stop=(st == N_ST - 1),
                            skip_group_check=True,
                        )

                    return th

                def epilogue_th():
                    # y [1, D] -> o = y @ W_v  (4 K=1 transposes + 4 matmuls)
                    y_sb = wvec_pool.tile([1, NCH], BF16, tag="ysb")
                    nc.scalar.copy(y_sb[:], y_ps[0:1, :])
                    for c in range(N_DT):
                        row_to_bcast_cols(
                            y_sb[0:1, c * P : (c + 1) * P],
                            yt_pads, c, f"ytp{c}", c % len(pad_ps),
                        )
                    for c in range(N_DT):
                        nc.tensor.matmul(
                            o_ps,
                            yt_pads[c][:],
                            wv_s[:, c, :],
                            start=(c == 0),
                            stop=(c == N_DT - 1),
                            skip_group_check=True,
                        )
                    o_sb = wvec_pool.tile([1, NCH], F32, tag="osb")
                    nc.scalar.copy(o_sb[:], o_ps[0:1, :])
                    nc.sync.dma_start(out=out_ext[b : b + 1, :], in_=o_sb[:])

                # pads lead the matvec by 6 so the K=1->broadcast chain
                # (2-bank pipelined) stays well ahead of the ymm stream
                LEAD = 6
                for kc in range(N_KC):
                    thunks.append(make_wcopy(kc))
                for kt in range(N_ST):
                    thunks.append(make_wtrans(kt))
                    if kt >= LEAD:
                        thunks.append(make_ymm(kt - LEAD))
                for st in range(N_ST - LEAD, N_ST):
                    thunks.append(make_ymm(st))
                thunks.append(epilogue_th)
                return thunks

            # ------------------------- emission ------------------------------

            # batch 0: M prework + transposes woven into the G projection
            # proj0 runs before any scores phase, so its ring can span all 8
            # PSUM banks; proj1 overlaps batch 0's w-phase (which owns the w
            # banks) and keeps the 4-bank ring.
            ring0 = bank_ring(with_w=True)
            h0, dma0, trans0, kq0 = proj_thunks(0, xnat0_s, x0_loaded, ring0)
            g0, xt0 = h0
            if dma0[0] is not None:
                dma0[0]()
                dma0[0] = None

            for th in trans0[0]:
                th()
            for th in m_prework_thunks(ring0):
                th()
            trans0 = [[], *trans0[1:]]
            emit_ltp(dma0, trans0, kq0)

            # batch 1's tiles + x loads kick off before batch 0's scores
            # phase (gpsimd + DMA idle there); its first transpose units fill
            # the PE stall at the end of the scores phase (the ring's lazy
            # PSUM alloc keeps the pool rotation aligned with emission time).
            xnat1_s = xnat_pool.tile([P, N_ST, D], BF16, tag="xnat")
            ring1 = bank_ring()
            h1, dma1, trans1, kq1 = proj_thunks(1, xnat1_s, [False] * N_SC, ring1)
            g1, xt1 = h1
            for sc in range(N_SC):
                if dma1[sc] is not None:
                    dma1[sc]()
                    dma1[sc] = None

            wps0 = phase_scores(0, g0, xt0, tail_filler=trans1[0])
            trans1 = [[], *trans1[1:]]

            # batch 1 projections with batch 0's w-phase woven in
            emit_ltp(dma1, trans1, kq1, extra=final_thunks(0, wps0, xnat0_s))

            # keep the PE warm through the final exp->r->colsum latency with
            # harmless dense matmuls (results unused)
            def dummy_filler():
                thunks = []

                def make(i):
                    def th():
                        pair_t = sc_pair()
                        for s in range(2):
                            nc.tensor.matmul(
                                pair_t[:, s, :],
                                xnat1_s[:, 2 * i + s, 0:P],
                                xnat1_s[:, 2 * i + s, :],
                                start=True,
                                stop=True,
                                skip_group_check=True,
                            )

                    return th

                for i in range(3):
                    thunks.append(make(i))
                return thunks

            wps1 = phase_scores(1, g1, xt1, tail_filler=dummy_filler())

            # w-phase pads for the last batch pipeline across all 4 scores
            # banks (free after the last exp)
            ring_w = bank_ring()
            pad_slots1 = [ring_w()[:, 0:1] for _ in range(4)]
            for th in final_thunks(1, wps1, xnat1_s, pad_ps=pad_slots1):
                th()

    nc.compile()
    return nc


_NC_CACHE = None


def _get_nc():
    global _NC_CACHE
    if _NC_CACHE is None:
        _NC_CACHE = build_nc()
    return _NC_CACHE


def make_in_maps(inputs, W_q, W_k, W_v):
    inputs = np.ascontiguousarray(np.asarray(inputs, dtype=np.float32))
    W_q = np.ascontiguousarray(np.asarray(W_q, dtype=np.float32))
    W_k = np.ascontiguousarray(np.asarray(W_k, dtype=np.float32))
    W_v = np.ascontiguousarray(np.asarray(W_v, dtype=np.float32))
    return [
        {
            "inputs": inputs[i * B_PER_CORE : (i + 1) * B_PER_CORE],
            "W_q": W_q,
            "W_k": W_k,
            "W_v": W_v,
        }
        for i in range(N_CORES)
    ]


def kernel(**inputs) -> np.ndarray:
    nc = _get_nc()
    in_maps = make_in_maps(
        inputs["inputs"], inputs["W_q"], inputs["W_k"], inputs["W_v"]
    )
    res = run_bass_kernel_spmd(nc, in_maps, core_ids=list(range(N_CORES)))
    return np.concatenate(
        [res.results[i]["out"] for i in range(N_CORES)], axis=0
    ).astype(np.float32)
